# revision 25
# baseline (speedup 1.0000x reference)
"""AttZip Llama streaming attention (decode step) for 8 trn2 NeuronCores.

Sharding: tensor-parallel over heads (4 heads/core).  Per core:
  - QKV projections for its head block + RoPE on the new token
  - streams its share of the KV cache once, computes scores (DVE),
    softmax (ACT exp + PE partition-sum), context (PE), output proj (PE)
  - importance partial (sum over local heads) + attn-out partial
    -> one 8-core AllReduce of a packed [8, 8193] buffer
  - topk-4 (DVE max8/max_index) + index sort, evicted-KV gathers
    (indirect DMA) and recent-window copies from SBUF.
Host glue shards inputs / concatenates head blocks of the outputs.
"""

import numpy as np

import concourse.bass as bass
import concourse.bacc as bacc
import concourse.mybir as mybir
import concourse.tile as tile
from concourse.masks import make_identity

F32 = mybir.dt.float32
I32 = mybir.dt.int32
U32 = mybir.dt.uint32

B, H, S, HD = 8, 32, 4096, 128
D = H * HD                      # 4096
NCORES = 8
HC = H // NCORES                # 4 local heads
NW = HC * HD                    # 512 = per-core projection width
KV = S + 1                      # 4097
RECENT = 512
IMP = 4
CAND = KV - RECENT              # 3585
NCH = S // 128                  # 32 s-chunks per (b,h)
BH = B * HC                     # 32 (b, local head) pairs per core
EV = IMP + RECENT               # 516
INVSQ = 1.0 / float(np.sqrt(HD))
ROPE_THETA = 10000.0

Exp = mybir.ActivationFunctionType.Exp
Alu = mybir.AluOpType


def _build_nc() -> bass.Bass:
    nc = bacc.Bacc("TRN2", target_bir_lowering=False,
                   num_devices=NCORES, num_swdge_queues=4)

    # ---- external inputs (per-core shards prepared by host) ----
    hiddenT = nc.dram_tensor("hiddenT", [D, B], F32, kind="ExternalInput")
    wq = nc.dram_tensor("wq", [D, NW], F32, kind="ExternalInput")
    wk = nc.dram_tensor("wk", [D, NW], F32, kind="ExternalInput")
    wv = nc.dram_tensor("wv", [D, NW], F32, kind="ExternalInput")
    wo = nc.dram_tensor("wo", [NW, D], F32, kind="ExternalInput")
    pk = nc.dram_tensor("pk", [BH * S, HD], F32, kind="ExternalInput")
    pv = nc.dram_tensor("pv", [BH * S, HD], F32, kind="ExternalInput")
    prevp = nc.dram_tensor("prevp", [B, KV], F32, kind="ExternalInput")
    cosr = nc.dram_tensor("cosr", [B, NW], F32, kind="ExternalInput")
    sinr = nc.dram_tensor("sinr", [B, NW], F32, kind="ExternalInput")
    base128 = nc.dram_tensor("base128", [128, 1], I32, kind="ExternalInput")

    # ---- external outputs ----
    attn_o = nc.dram_tensor("attn_o", [B, D], F32, kind="ExternalOutput")
    kev_o = nc.dram_tensor("kev_o", [B, HC, EV, HD], F32, kind="ExternalOutput")
    vev_o = nc.dram_tensor("vev_o", [B, HC, EV, HD], F32, kind="ExternalOutput")
    impk_o = nc.dram_tensor("impk_o", [B, EV], F32, kind="ExternalOutput")

    from contextlib import ExitStack

    with tile.TileContext(nc) as tc:
        with (
            tc.tile_pool(name="cpool", bufs=1) as cpool,
            tc.tile_pool(name="psum", bufs=2, space="PSUM") as psum,
            tc.tile_pool(name="dram", bufs=1, space="DRAM") as dram,
        ):
            inner = ExitStack()
            wpool = inner.enter_context(tc.tile_pool(name="wpool", bufs=2))
            kvpool = inner.enter_context(tc.tile_pool(name="kvpool", bufs=3))
            qrpool = inner.enter_context(tc.tile_pool(name="qrpool", bufs=2))
            scpool = inner.enter_context(tc.tile_pool(name="scpool", bufs=2))
            # ---------- constants ----------
            ident = cpool.tile([128, 128], F32)
            make_identity(nc, ident[:])
            ones_k1 = cpool.tile([1, 128], F32)
            nc.vector.memset(ones_k1[:], 1.0)
            ones_p = cpool.tile([128, 1], F32)
            nc.vector.memset(ones_p[:], 1.0)
            cos_sb = cpool.tile([B, NW], F32)
            nc.gpsimd.dma_start(out=cos_sb[:], in_=cosr[:])
            sin_sb = cpool.tile([B, NW], F32)
            nc.gpsimd.dma_start(out=sin_sb[:], in_=sinr[:])
            ht_sb = cpool.tile([128, D // 128, B], F32)
            nc.gpsimd.dma_start(
                out=ht_sb[:], in_=hiddenT[:].rearrange("(c p) b -> p c b", p=128)
            )
            base_sb = cpool.tile([128, 1], I32)
            nc.gpsimd.dma_start(out=base_sb[:], in_=base128[:])
            # zero stationary rows for zero-matmuls (see zt() below) and a
            # junk rhs whose only job is carrying DVE-ordering deps
            zero128 = cpool.tile([1, 128], F32)
            nc.vector.memset(zero128[:], 0.0)
            junk512 = cpool.tile([1, 512], F32)
            nc.vector.memset(junk512[:], 0.0)

            # ---------- QKV projections ----------
            def pe_touch(ap):
                # absorb a DMA-queue wait into a throwaway 1x1 matmul so the
                # next real matmul (LDW slot: 1 wait max) stays legal
                tch = psum.tile([1, 1], F32, tag="tch", name="tch", bufs=1)
                nc.tensor.matmul(out=tch[:], lhsT=ap, rhs=ap, start=True, stop=True)

            q_sb = cpool.tile([B, NW], F32)
            kn_sb = cpool.tile([B, NW], F32)
            vn_sb = cpool.tile([B, NW], F32)

            def half(ap, which):
                # [B, NW] -> [B, HC, 64] view of low (0) / high (1) half of
                # each head's 128 dims
                return bass.AP(
                    tensor=ap.tensor,
                    offset=ap.offset + which * 64,
                    ap=[ap.ap[0], [128, HC], [1, 64]],
                )

            pe_touch(ht_sb[0:1, 0, 0:1])
            for wdr, dst, do_rope in ((wq, q_sb, True), (wk, kn_sb, True), (wv, vn_sb, False)):
                ps = psum.tile([B, NW], F32, tag="qkvps", name="qkvps")
                # zero matmul opens the accumulation group: clears has_written
                # over the full tile while absorbing the PSUM-slot wait, so
                # each later matmul carries at most one (DMA) wait
                nc.tensor.matmul(
                    out=ps[:], lhsT=zero128[:, 0:B], rhs=junk512[:, 0:NW],
                    start=True, stop=False,
                )
                for kc in range(4):
                    wch = wpool.tile([128, 8, NW], F32, tag="wch", name="wch")
                    nc.gpsimd.dma_start(
                        out=wch[:],
                        in_=wdr[kc * 1024 : (kc + 1) * 1024, :].rearrange(
                            "(c p) n -> p c n", p=128
                        ),
                    )
                    for c8 in range(8):
                        nc.tensor.matmul(
                            out=ps[:],
                            lhsT=ht_sb[:, kc * 8 + c8, :],
                            rhs=wch[:, c8, :],
                            start=False,
                            stop=(kc == 3 and c8 == 7),
                        )
                if do_rope:
                    tcos = scpool.tile([B, NW], F32, tag="tcos", name="tcos")
                    nc.vector.tensor_tensor(out=tcos[:], in0=ps[:], in1=cos_sb[:], op=Alu.mult)
                    tmp = scpool.tile([B, HC, 64], F32, tag="rtmp", name="rtmp")
                    # low half: q*cos - q_hi*sin
                    nc.vector.tensor_tensor(
                        out=tmp[:], in0=half(ps[:], 1), in1=half(sin_sb[:], 0), op=Alu.mult
                    )
                    nc.vector.tensor_tensor(
                        out=half(dst[:], 0), in0=half(tcos[:], 0), in1=tmp[:], op=Alu.subtract
                    )
                    # high half: q*cos + q_lo*sin
                    tmp2 = scpool.tile([B, HC, 64], F32, tag="rtmp", name="rtmp2")
                    nc.vector.tensor_tensor(
                        out=tmp2[:], in0=half(ps[:], 0), in1=half(sin_sb[:], 1), op=Alu.mult
                    )
                    nc.vector.tensor_tensor(
                        out=half(dst[:], 1), in0=half(tcos[:], 1), in1=tmp2[:], op=Alu.add
                    )
                else:
                    nc.vector.tensor_copy(out=dst[:], in_=ps[:])

            # ---------- new-token scores (exp'd) for every (b, hl) ----------
            nst = cpool.tile([B, HC], F32)       # raw scores, partition b
            ens = cpool.tile([B, HC], F32)       # exp(scores), partition b
            nprod = scpool.tile([B, NW], F32, tag="nprod")
            nc.vector.scalar_tensor_tensor(
                out=nprod[:], in0=kn_sb[:], scalar=INVSQ, in1=q_sb[:],
                op0=Alu.mult, op1=Alu.mult,
            )
            nc.vector.reduce_sum(
                out=nst[:],
                in_=nprod[:].rearrange("b (h d) -> b h d", h=HC),
                axis=mybir.AxisListType.X,
            )
            nc.scalar.activation(out=ens[:], in_=nst[:], func=Exp)
            # bounce [B, HC] (partition-major) -> [1, BH] (free-major) via DRAM
            ens_dram = dram.tile([B, HC], F32)
            nc.gpsimd.dma_start(out=ens_dram[:], in_=ens[:])
            ens_r = cpool.tile([1, BH], F32)
            nc.gpsimd.dma_start(
                out=ens_r[:], in_=ens_dram[:].rearrange("b h -> (b h)")[None, :]
            )
            # broadcast exp(new scores) across all partitions (0-step DMA read)
            ens_rep = cpool.tile([128, BH], F32)
            nc.gpsimd.dma_start(
                out=ens_rep[:],
                in_=bass.AP(tensor=ens_dram[:].tensor, offset=ens_dram[:].offset,
                            ap=[[0, 128], [1, BH]]),
            )
            # v_new transposed to [d, b] per local head -> [128, BH] (d, (b,hl))
            vnt_sb = cpool.tile([128, BH], F32)
            vnt_v = vnt_sb[:].rearrange("p (b h) -> p b h", h=HC)
            for hl in range(HC):
                tvps = psum.tile([128, B], F32, tag="aux", name="tvps", bufs=1)
                nc.tensor.transpose(
                    out=tvps[:], in_=vn_sb[:, hl * HD : (hl + 1) * HD],
                    identity=ident[0:B, 0:B],
                )
                nc.scalar.copy(out=vnt_v[:, :, hl], in_=tvps[:])
            # q rows staged to DRAM for per-b partition-broadcast reads
            q_dram = dram.tile([B, NW], F32)
            nc.gpsimd.dma_start(out=q_dram[:], in_=q_sb[:])

            # ---------- main attention loop ----------
            exp_all = cpool.tile([128, BH * NCH], F32)   # unnormalized attn weights
            dparts = cpool.tile([128, BH], F32)          # per-partition denom partials
            ctx_raw = cpool.tile([128, BH], F32)         # unnormalized ctx, d on partitions
            impa = [cpool.tile([128, NCH], F32, name=f"impa{b}") for b in range(B)]

            for b in range(B):
                # replicate q row b across all 128 partitions (0-step DMA read)
                q_rep = qrpool.tile([128, NW], F32, tag="qrep", name="qrep")
                nc.gpsimd.dma_start(
                    out=q_rep[:],
                    in_=bass.AP(tensor=q_dram[:].tensor,
                                offset=q_dram[:].offset + b * NW,
                                ap=[[0, 128], [1, NW]]),
                )
                for hl in range(HC):
                    bh = b * HC + hl
                    cs = slice(hl * HD, (hl + 1) * HD)
                    ksb = kvpool.tile([128, NCH, HD], F32, tag="ksb", name="ksb")
                    vsb = kvpool.tile([128, NCH, HD], F32, tag="vsb", name="vsb")
                    nc.gpsimd.dma_start(
                        out=ksb[:],
                        in_=pk[bh * S : (bh + 1) * S, :].rearrange("(c p) d -> p c d", p=128),
                    )
                    nc.gpsimd.dma_start(
                        out=vsb[:],
                        in_=pv[bh * S : (bh + 1) * S, :].rearrange("(c p) d -> p c d", p=128),
                    )
                    # scores: per chunk, accum over d of (k/sqrt(hd)) * q
                    scores = scpool.tile([128, NCH], F32, tag="scores", name="scores")
                    scratch = scpool.tile([128, HD], F32, tag="scratch", name="scratch")
                    for c in range(NCH):
                        nc.vector.scalar_tensor_tensor(
                            out=scratch[:],
                            in0=ksb[:, c, :],
                            scalar=INVSQ,
                            in1=q_rep[:, cs],
                            op0=Alu.mult,
                            op1=Alu.mult,
                            accum_out=scores[:, c : c + 1],
                        )
                    # exp + per-partition denominator partial
                    nc.scalar.activation(
                        out=exp_all[:, bh * NCH : (bh + 1) * NCH],
                        in_=scores[:],
                        func=Exp,
                        accum_out=dparts[:, bh : bh + 1],
                    )
                    # context: ctx[d] = sum_s exp[s] * v[s, d], d on partitions
                    cps = psum.tile([128, 1], F32, tag="ctxps", name="ctxps")
                    nc.tensor.matmul(
                        out=cps[:],
                        lhsT=zero128[:],
                        rhs=exp_all[0:1, bh * NCH : bh * NCH + 1],
                        start=True, stop=False,
                    )
                    for c in range(NCH):
                        nc.tensor.matmul(
                            out=cps[:],
                            lhsT=vsb[:, c, :],
                            rhs=exp_all[:, bh * NCH + c : bh * NCH + c + 1],
                            start=False,
                            stop=(c == NCH - 1),
                        )
                    nc.scalar.copy(out=ctx_raw[:, bh : bh + 1], in_=cps[:])
                    # eviction: recent window.  rows 4..130 (s 3585..3711)
                    # via DRAM->DRAM (SBUF APs cannot start at partition 1)
                    nc.gpsimd.dma_start(
                        out=kev_o[:][b, hl, 4:131, :],
                        in_=pk[bh * S + 3585 : bh * S + 3712, :],
                    )
                    nc.gpsimd.dma_start(
                        out=vev_o[:][b, hl, 4:131, :],
                        in_=pv[bh * S + 3585 : bh * S + 3712, :],
                    )
                    # rows 131..514 <- s 3712..4095 (chunks 29..31)
                    nc.gpsimd.dma_start(
                        out=kev_o[:][b, hl, 131:515, :].rearrange("(c p) d -> p c d", p=128),
                        in_=ksb[:, 29:32, :],
                    )
                    nc.gpsimd.dma_start(
                        out=vev_o[:][b, hl, 131:515, :].rearrange("(c p) d -> p c d", p=128),
                        in_=vsb[:, 29:32, :],
                    )

            # new-token row 515 of the evicted caches (all (b,hl) at once)
            nc.gpsimd.dma_start(
                out=kev_o[:][:, :, 515, :], in_=kn_sb[:].rearrange("b (h d) -> b h d", h=HC)
            )
            nc.gpsimd.dma_start(
                out=vev_o[:][:, :, 515, :], in_=vn_sb[:].rearrange("b (h d) -> b h d", h=HC)
            )

            # ---------- denominators / reciprocals ----------
            dps = psum.tile([1, BH], F32, tag="aux", name="dps", bufs=1)
            nc.tensor.matmul(out=dps[:], lhsT=ones_p[:], rhs=dparts[:], start=True, stop=True)
            denom = cpool.tile([1, BH], F32)
            nc.vector.tensor_tensor(out=denom[:], in0=dps[:], in1=ens_r[:], op=Alu.add)
            recips = cpool.tile([1, BH], F32)
            nc.vector.reciprocal(out=recips[:], in_=denom[:])
            rr_ps = psum.tile([128, BH], F32, tag="aux", name="rrps", bufs=1)
            nc.tensor.matmul(out=rr_ps[:], lhsT=ones_k1[:], rhs=recips[:], start=True, stop=True)
            recip_rep = cpool.tile([128, BH], F32)
            nc.vector.tensor_copy(out=recip_rep[:], in_=rr_ps[:])

            ar_in = dram.tile([B, 4096 + KV], F32)
            ar_out = dram.tile([B, 4096 + KV], F32, addr_space="Shared")

            # ---------- add new-token term; normalize ctx; output projection ----------
            ntk = cpool.tile([128, BH], F32)
            nc.vector.tensor_tensor(out=ntk[:], in0=vnt_sb[:], in1=ens_rep[:], op=Alu.mult)
            nc.vector.tensor_tensor(out=ctx_raw[:], in0=ctx_raw[:], in1=ntk[:], op=Alu.add)
            ctx_norm = cpool.tile([128, BH], F32)
            cr_v = ctx_raw[:].rearrange("p (b h) -> p b h", h=HC)
            cn_v = ctx_norm[:].rearrange("p (b h) -> p b h", h=HC)
            rr_v = recip_rep[:].rearrange("p (b h) -> p b h", h=HC)
            for hl in range(HC):
                nc.vector.tensor_tensor(
                    out=cn_v[:, :, hl], in0=cr_v[:, :, hl], in1=rr_v[:, :, hl], op=Alu.mult
                )
            nc.vector.tensor_copy(out=junk512[:, 0:BH], in_=ctx_norm[0:1, :])
            for n in range(8):
                woch = wpool.tile([128, HC, 512], F32, tag="wch", name="woch")
                nc.gpsimd.dma_start(
                    out=woch[:],
                    in_=wo[:, n * 512 : (n + 1) * 512].rearrange("(h p) m -> p h m", p=128),
                )
                wops = psum.tile([B, 512], F32, tag="wops", name="wops")
                nc.tensor.matmul(
                    out=wops[:], lhsT=zero128[:, 0:B], rhs=junk512[:],
                    start=True, stop=False,
                )
                for hl in range(HC):
                    nc.tensor.matmul(
                        out=wops[:],
                        lhsT=cn_v[:, :, hl],
                        rhs=woch[:, hl, :],
                        start=False,
                        stop=(hl == HC - 1),
                    )
                wcp = scpool.tile([B, 512], F32, tag="wcp", name="wcp")
                nc.scalar.copy(out=wcp[:], in_=wops[:])
                nc.gpsimd.dma_start(
                    out=ar_in[:][:, n * 512 : (n + 1) * 512], in_=wcp[:]
                )

            # ---------- importance partials ----------
            for b in range(B):
                nc.vector.memset(impa[b][:], 0.0)
                for hl in range(HC):
                    bh = b * HC + hl
                    nc.vector.scalar_tensor_tensor(
                        out=impa[b][:],
                        in0=exp_all[:, bh * NCH : (bh + 1) * NCH],
                        scalar=recip_rep[:, bh : bh + 1],
                        in1=impa[b][:],
                        op0=Alu.mult,
                        op1=Alu.add,
                    )
            # new-token importance: sum over local heads of exp/denom
            impnew = cpool.tile([1, BH], F32)
            nc.vector.tensor_tensor(out=impnew[:], in0=ens_r[:], in1=recips[:], op=Alu.mult)
            impnb = cpool.tile([1, B], F32)
            nc.vector.reduce_sum(
                out=impnb[:],
                in_=impnew[:].rearrange("p (b h) -> p b h", h=HC),
                axis=mybir.AxisListType.X,
            )

            # ---------- pack + all-reduce ----------
            for b in range(B):
                tps = psum.tile([NCH, 128], F32, tag="aux", name="tps", bufs=1)
                nc.tensor.transpose(out=tps[:], in_=impa[b][:], identity=ident[:])
                impT = scpool.tile([NCH, 128], F32, tag="impT", name="impT")
                nc.vector.tensor_copy(out=impT[:], in_=tps[:])
                nc.gpsimd.dma_start(
                    out=ar_in[:][b, 4096:8192].rearrange("(c d) -> c d", d=128),
                    in_=impT[:],
                )
            nc.gpsimd.dma_start(
                out=ar_in[:][:, 8192:8193].rearrange("b o -> o b"), in_=impnb[:]
            )
            inner.close()
            post = inner.enter_context(tc.tile_pool(name="post", bufs=1))
            nc.gpsimd.collective_compute(
                "AllReduce",
                Alu.add,
                replica_groups=[list(range(NCORES))],
                ins=[ar_in.opt()],
                outs=[ar_out.opt()],
            )

            # ---------- attn output ----------
            nc.gpsimd.dma_start(out=attn_o[:], in_=ar_out[:][:, 0:4096])

            # ---------- final importance ----------
            imp_sb = post.tile([B, KV], F32)
            nc.gpsimd.dma_start(out=imp_sb[:], in_=ar_out[:][:, 4096 : 4096 + KV])
            prev_sb = post.tile([B, KV], F32)
            nc.gpsimd.dma_start(out=prev_sb[:], in_=prevp[:])
            nc.vector.scalar_tensor_tensor(
                out=imp_sb[:], in0=imp_sb[:], scalar=1.0 / H, in1=prev_sb[:],
                op0=Alu.mult, op1=Alu.add,
            )
            nc.gpsimd.dma_start(out=impk_o[:][:, IMP:EV], in_=imp_sb[:, CAND:KV])

            # ---------- top-4 of imp[:, :CAND], sorted by index ----------
            vals8 = post.tile([B, 8], F32)
            nc.vector.max(out=vals8[:], in_=imp_sb[:, 0:CAND])
            idx8 = post.tile([B, 8], U32)
            nc.vector.max_index(out=idx8[:], in_max=vals8[:], in_values=imp_sb[:, 0:CAND])
            idxf = post.tile([B, IMP], F32)
            nc.vector.tensor_copy(out=idxf[:], in_=idx8[:, 0:IMP])
            vals = post.tile([B, IMP], F32)
            nc.vector.tensor_copy(out=vals[:], in_=vals8[:, 0:IMP])
            cond = post.tile([B, 1], I32)
            mn = post.tile([B, 1], F32)
            mx = post.tile([B, 1], F32)
            tv = post.tile([B, 1], F32)
            for i, j in ((0, 1), (2, 3), (0, 2), (1, 3), (1, 2)):
                ii = slice(i, i + 1)
                jj = slice(j, j + 1)
                nc.vector.tensor_tensor(out=cond[:], in0=idxf[:, ii], in1=idxf[:, jj], op=Alu.is_gt)
                nc.vector.tensor_tensor(out=mn[:], in0=idxf[:, ii], in1=idxf[:, jj], op=Alu.min)
                nc.vector.tensor_tensor(out=mx[:], in0=idxf[:, ii], in1=idxf[:, jj], op=Alu.max)
                nc.vector.tensor_copy(out=idxf[:, ii], in_=mn[:])
                nc.vector.tensor_copy(out=idxf[:, jj], in_=mx[:])
                nc.vector.tensor_copy(out=tv[:], in_=vals[:, ii])
                nc.vector.copy_predicated(vals[:, ii], cond[:], vals[:, jj])
                nc.vector.copy_predicated(vals[:, jj], cond[:], tv[:])
            nc.gpsimd.dma_start(out=impk_o[:][:, 0:IMP], in_=vals[:])

            # ---------- evicted-KV gathers for the top-4 rows ----------
            idxi = post.tile([B, IMP], I32)
            nc.vector.tensor_copy(out=idxi[:], in_=idxf[:])
            rep16 = post.tile([B, HC, IMP], I32)
            rep_src = bass.AP(
                tensor=idxi[:].tensor,
                offset=idxi[:].offset,
                ap=[idxi[:].ap[0], [0, HC], [1, IMP]],
            )
            nc.vector.tensor_copy(out=rep16[:], in_=rep_src)
            idx_scr = dram.tile([128, 1], I32)
            nc.gpsimd.dma_start(
                out=idx_scr[:].rearrange("(a b) o -> a (b o)", a=B), in_=rep16[:]
            )
            idx128 = post.tile([128, 1], I32)
            nc.gpsimd.dma_start(out=idx128[:], in_=idx_scr[:])
            gidx = post.tile([128, 1], I32)
            nc.vector.tensor_tensor(out=gidx[:], in0=idx128[:], in1=base_sb[:], op=Alu.add)
            kg = post.tile([128, HD], F32)
            nc.gpsimd.indirect_dma_start(
                out=kg[:],
                out_offset=None,
                in_=pk[:],
                in_offset=bass.IndirectOffsetOnAxis(ap=gidx[:, 0:1], axis=0),
            )
            vg = post.tile([128, HD], F32)
            nc.gpsimd.indirect_dma_start(
                out=vg[:],
                out_offset=None,
                in_=pv[:],
                in_offset=bass.IndirectOffsetOnAxis(ap=gidx[:, 0:1], axis=0),
            )
            # SBUF rows (b, h, j) land on kev[b, h, j, :] — same linearization
            nc.gpsimd.dma_start(out=kev_o[:][:, :, 0:IMP, :], in_=kg[:])
            nc.gpsimd.dma_start(out=vev_o[:][:, :, 0:IMP, :], in_=vg[:])
            inner.close()

    nc.compile()
    return nc


_CACHE: dict = {}


def _get_nc() -> bass.Bass:
    if "nc" not in _CACHE:
        _CACHE["nc"] = _build_nc()
    return _CACHE["nc"]


def _host_inputs(hidden_states, past_key, past_value, Wq, Wk, Wv, Wo, imp_score_prev):
    hidden = np.asarray(hidden_states, dtype=np.float32).reshape(B, D)
    hiddenT = np.ascontiguousarray(hidden.T)
    Wq = np.asarray(Wq, dtype=np.float32)
    Wk = np.asarray(Wk, dtype=np.float32)
    Wv = np.asarray(Wv, dtype=np.float32)
    Wo = np.asarray(Wo, dtype=np.float32)
    past_key = np.asarray(past_key, dtype=np.float32)
    past_value = np.asarray(past_value, dtype=np.float32)
    prev = np.asarray(imp_score_prev, dtype=np.float32)

    # bit-exact replica of the reference's f32 RoPE table (jax f32 pow ==
    # correctly-rounded f64 pow; jax f32 cos/sin ~ f64-accurate)
    x = np.arange(0, HD, 2, dtype=np.float32) / np.float32(HD)
    p = (ROPE_THETA ** x.astype(np.float64)).astype(np.float32)
    inv_freq = (np.float32(1.0) / p).astype(np.float32)
    ang = (np.float32(S) * inv_freq).astype(np.float32)
    cos1 = np.cos(ang.astype(np.float64)).astype(np.float32)
    sin1 = np.sin(ang.astype(np.float64)).astype(np.float32)
    cosf = np.concatenate([cos1, cos1]).astype(np.float32)
    sinf = np.concatenate([sin1, sin1]).astype(np.float32)
    cos_rep = np.ascontiguousarray(np.broadcast_to(np.tile(cosf, HC)[None, :], (B, NW)))
    sin_rep = np.ascontiguousarray(np.broadcast_to(np.tile(sinf, HC)[None, :], (B, NW)))
    prevp = np.concatenate([prev, np.zeros((B, 1), np.float32)], axis=1)
    base = ((np.arange(128, dtype=np.int64) // IMP) * S).astype(np.int32)[:, None]

    shared = {
        "hiddenT": hiddenT,
        "cosr": cos_rep,
        "sinr": sin_rep,
        "prevp": np.ascontiguousarray(prevp),
        "base128": base,
    }
    in_maps = []
    for c in range(NCORES):
        hs = slice(c * HC, (c + 1) * HC)
        cs = slice(c * NW, (c + 1) * NW)
        in_maps.append(
            dict(
                shared,
                wq=np.ascontiguousarray(Wq[:, cs]),
                wk=np.ascontiguousarray(Wk[:, cs]),
                wv=np.ascontiguousarray(Wv[:, cs]),
                wo=np.ascontiguousarray(Wo[cs, :]),
                pk=np.ascontiguousarray(past_key[:, hs]).reshape(BH * S, HD),
                pv=np.ascontiguousarray(past_value[:, hs]).reshape(BH * S, HD),
            )
        )
    return in_maps


def _assemble(results):
    attn = np.asarray(results[0]["attn_o"]).reshape(B, 1, D).copy()
    k_ev = np.concatenate(
        [np.asarray(r["kev_o"]).reshape(B, HC, EV, HD) for r in results], axis=1
    )
    v_ev = np.concatenate(
        [np.asarray(r["vev_o"]).reshape(B, HC, EV, HD) for r in results], axis=1
    )
    imp_kept = np.asarray(results[0]["impk_o"]).reshape(B, EV).copy()
    return attn, k_ev, v_ev, imp_kept


_TRACE = False       # set by test.py to capture an NTFF profile
_LAST_RESULT = None  # BassKernelResults of the most recent run (for test.py)


def kernel(hidden_states, past_key, past_value, Wq, Wk, Wv, Wo, imp_score_prev):
    from concourse.bass_utils import run_bass_kernel_spmd

    global _LAST_RESULT
    nc = _get_nc()
    in_maps = _host_inputs(
        hidden_states, past_key, past_value, Wq, Wk, Wv, Wo, imp_score_prev
    )
    res = run_bass_kernel_spmd(
        nc, in_maps, core_ids=list(range(NCORES)), trace=_TRACE
    )
    _LAST_RESULT = res
    return _assemble(res.results)
